# revision 20
# baseline (speedup 1.0000x reference)
"""VQ codebook assignment kernel for Trainium2 (8 NeuronCores).

Problem: X (8,4096,128) f32, centroids (1024,128), mean/scale (128,),
mask (8,4096). Output: one-hot C (8,4096,1024) f32 of the nearest
centroid (L2 over standardized points), times mask.

Strategy (data-parallel, core b owns batch b):
  argmin_k ||(x-mean)/scale - c_k||^2 == argmax_k [ x . (c_k/scale) - b_k ]
  with b_k = mean . (c_k/scale) + ||c_k||^2 / 2.
  Scores are computed on the PE with a 3-term fp16 split matmul
  (Xh@Ch + Xh@Cl + Xl@Ch, fp32 PSUM accumulation); fp16 products are
  exact in f32, so every argmax matches the f32 reference.
  The -b_k bias enters the same PSUM accumulation via a rank-2 fp16
  matmul (ones2 x 2-way fp16 split of -b).  b itself is computed as the
  partition-axis sum (GPSIMD) of u = c^T .* (0.5 c^T + (mean/scale)) --
  no PE or DMA involvement in bias prep.
  One-hot extraction: m = rowmax(scores) on DVE, then on ACT
  out = Exp(scores * 2^100 - m * 2^100): exactly 1.0 at the argmax and
  exactly 0.0 elsewhere.  Zero mask rows get ln(mask) (= -inf) added to
  the bias (computed on GPSIMD).

Layouts (all chosen for large contiguous DMA descriptors):
  X loads as xall[p, s*128+d] = X[32p+s, d] in four 512 KiB DMAs
  (4 KiB contiguous per partition each).  Point tile t =
  xall[:, t*128:(t+1)*128] holds points {32p+t}; the output rows for
  tile t are written with a partition-stride-32 AP (4 KiB contiguous
  descriptors).  Centroids load as ck[p, e*128+d] = c[8p+e, d] (4 KiB
  contiguous); chunk-e transposes write back with a stride-8 column AP
  so centT/ch/cl end up in canonical k order; ch/cl/u are produced
  per-chunk so the first score matmuls never wait on full-width ops.

Pipelining: x transposes run AHEAD tiles in front of their score
matmuls (2 ps_x PSUM slots), 3 score PSUM slots, output stores
alternate gpsimd/sync DMA queues.
"""
import numpy as np

import concourse.bass as bass
import concourse.bacc as bacc
import concourse.mybir as mybir
import concourse.tile as tile
from concourse import masks
from concourse.bass_utils import run_bass_kernel_spmd

B, N, D, K = 8, 4096, 128, 1024
PT = 128           # points per tile
NT = N // PT       # tiles per core
NC_ = K // 128     # centroid chunks
XQ = 4             # x load split
F32 = mybir.dt.float32
F16 = mybir.dt.float16
AF = mybir.ActivationFunctionType
OP = mybir.AluOpType
BIG = 2.0 ** 100
AHEAD = 2          # x-transpose pipelining depth (ps_x slots)


def _body(nc, tc, x_in, mask_in, c_in, mean_in, scale_in, out):
    import contextlib
    with contextlib.ExitStack() as ctx:
        # PSUM map: ps_s = 3 x [128,1024] score slots (6 banks; also setup
        # scratch -- setup finishes before slot reuse); ps_x = 2 x [128,128]
        # slots (2 banks) exclusively for the pipelined X transposes.
        ps_s = ctx.enter_context(tc.tile_pool(name="ps_s", bufs=3, space="PSUM"))
        ps_x = ctx.enter_context(tc.tile_pool(name="ps_x", bufs=AHEAD, space="PSUM"))

        sb = ctx.enter_context(tc.tile_pool(name="setup_sb", bufs=1))
        const = ctx.enter_context(tc.tile_pool(name="const", bufs=1))
        xs_pool = ctx.enter_context(tc.tile_pool(name="xs", bufs=12))
        mcol_pool = ctx.enter_context(tc.tile_pool(name="mcol", bufs=4))
        oh_pool = ctx.enter_context(tc.tile_pool(name="oh", bufs=4))

        _pset_n = [0]

        def pset(shape, dt):
            # setup PSUM scratch: borrow a score slot (partial use of the
            # [128,1024]-sized "sc"-tag slots; adds no PSUM footprint)
            _pset_n[0] += 1
            return ps_s.tile(shape, dt, name=f"pset{_pset_n[0]}", tag="sc")

        # centroids first: their DMA latency gates the bias-prep chain;
        # then the first quarter of X (tiles 0-7), then the rest.
        ck_all = sb.tile([128, K], F32)
        nc.sync.dma_start(ck_all[:], c_in[:].rearrange("(p e) d -> p (e d)", p=128))

        ms = sb.tile([2, 128], F32)
        nc.sync.dma_start(ms[0:1, :], mean_in[:].unsqueeze(0))
        nc.sync.dma_start(ms[1:2, :], scale_in[:].unsqueeze(0))
        maskc = sb.tile([128, NT], F32)
        nc.sync.dma_start(maskc[:], mask_in[:].rearrange("(p s) -> p s", p=128))

        xall = sb.tile([128, N], F32)
        xr = x_in[:].rearrange("(p s) d -> p (s d)", p=128)
        nc.sync.dma_start(xall[:, 0:N // XQ], xr[:, 0:N // XQ])
        for q in range(1, XQ):
            nc.sync.dma_start(xall[:, q * (N // XQ):(q + 1) * (N // XQ)],
                              xr[:, q * (N // XQ):(q + 1) * (N // XQ)])

        ident = const.tile([128, 128], F32)
        masks.make_identity(nc, ident[:])

        msT = const.tile([128, 2], F32)
        lnmask = const.tile([128, NT], F32)
        centT = sb.tile([128, K], F32)     # raw centroids^T [d, k], canonical k
        centT3 = centT[:].rearrange("d (p e) -> d p e", e=NC_)
        negb2 = const.tile([2, K], F16)

        # ln(mask): 0 for mask==1, -inf for mask==0.  lnmask[p, t] belongs to
        # point 32p+t, which is exactly tile t's partition p -- no transpose.
        nc.scalar.activation(lnmask[:], maskc[:], AF.Ln)

        p_ms = pset([128, 128], F32)
        nc.tensor.transpose(p_ms[:, 0:2], ms[:], ident[0:2, 0:2])
        nc.scalar.activation(msT[:], p_ms[:, 0:2], AF.Copy)
        recip = const.tile([128, 1], F32)
        nc.vector.reciprocal(recip[:], msT[:, 1:2])
        nmos = sb.tile([128, 1], F32)      # -mean/scale
        nc.vector.tensor_tensor(nmos[:], msT[:, 0:1], recip[:], op=OP.mult)
        nc.vector.tensor_scalar(nmos[:], nmos[:], -1.0, None, op0=OP.mult)

        # warm the PE HAM clock gate while the centroid DMA is in flight:
        # ~2.6us of dummy transposes lifts the array to 2.4 GHz before the
        # real setup matmuls (and the first score tiles) issue.
        for w in range(10):
            dmy = ps_x.tile([128, 128], F32, tag="x", name=f"warm{w}")
            nc.tensor.transpose(dmy[:], ident[:], ident[:])

        # chunk e of ck_all holds centroids {8p+e}; its transpose lands in
        # canonical columns {8p+e} via stride-8 APs (copies alternate
        # ACT/DVE).  Elementwise prep runs DENSE afterwards: strided
        # single-element inner dims cost ~3x on DVE/ACT.
        for e in range(NC_):
            p_ct = pset([128, 128], F32)
            nc.tensor.transpose(p_ct[:], ck_all[:, bass.ts(e, 128)], ident[:])
            if e % 2 == 0:
                nc.scalar.activation(centT3[:, :, e], p_ct[:], AF.Copy)
            else:
                nc.vector.tensor_copy(centT3[:, :, e], p_ct[:])

        # c' = c/scale; fp16 split ch+cl; nu = c .* (-0.5 c - mean/scale)
        # whose colsum is -b_k directly (negation folded into the integrand).
        # DVE order: cp, nta, nu (feeds the bias path ASAP), then cl.
        nta = sb.tile([128, K], F32)
        nc.vector.tensor_scalar(nta[:], centT[:], -0.5, nmos[:],
                                op0=OP.mult, op1=OP.add)
        nu = sb.tile([128, K], F32)
        nc.vector.tensor_tensor(nu[:], centT[:], nta[:], op=OP.mult)
        cpv = sb.tile([128, K], F32)
        nc.vector.tensor_scalar(cpv[:], centT[:], recip[:], None, op0=OP.mult)
        ch = const.tile([128, K], F16)
        nc.scalar.activation(ch[:], cpv[:], AF.Copy)
        cl = const.tile([128, K], F16)
        nc.vector.tensor_tensor(cl[:], cpv[:], ch[:], op=OP.subtract)

        # -b row = colsum(nu) via two matmuls with a single ones column as
        # stationary (one 1-col LDWEIGHTS; nu streams as moving operand);
        # each half is split fp16 as soon as its PSUM row lands (f32 row
        # copy on ACT runs parallel to the DVE fp16 cast).
        onecol = sb.tile([128, 1], F32)
        nc.vector.memset(onecol[:], 1.0)
        nbrow = sb.tile([1, K], F32)
        b1row = sb.tile([1, K], F16)
        for h in range(2):
            p_b = pset([1, 512], F32)
            nc.tensor.matmul(p_b[:], onecol[:], nu[:, bass.ts(h, 512)],
                             start=True, stop=True)
            nc.vector.tensor_copy(negb2[0:1, bass.ts(h, 512)], p_b[:])
            nc.scalar.activation(nbrow[0:1, bass.ts(h, 512)], p_b[:], AF.Copy)
            nc.vector.tensor_tensor(b1row[0:1, bass.ts(h, 512)],
                                    nbrow[0:1, bass.ts(h, 512)],
                                    negb2[0:1, bass.ts(h, 512)],
                                    op=OP.subtract)
        # compute engines cannot write at a partition offset; row 1 of negb2
        # is filled via DMA on the (idle) scalar HWDGE queue so it never
        # queues behind the X loads on the sync ring.
        nc.sync.dma_start(negb2[1:2, :], b1row[:])

        ones2 = const.tile([2, 128], F16)
        nc.vector.memset(ones2[:], 1.0)

        # output rows for tile t are {32p+t}: partition-stride-32 AP,
        # each descriptor one contiguous 4 KiB row
        out_t = out.rearrange("(p s) k -> s p k", s=NT)

        xs_tiles = {}

        def x_prep(t):
            # PE transpose of x tile t, then fp16 hi/lo split out of PSUM
            # (hi on ACT, lo on DVE)
            xt = ps_x.tile([128, PT], F32, tag="x")
            nc.tensor.transpose(xt[:], xall[:, bass.ts(t, PT)], ident[:])
            xh = xs_pool.tile([128, PT], F16, tag="xh")
            nc.scalar.activation(xh[:], xt[:], AF.Copy)
            xl = xs_pool.tile([128, PT], F16, tag="xl")
            nc.vector.tensor_tensor(xl[:], xt[:], xh[:], op=OP.subtract)
            xs_tiles[t] = (xh, xl)

        for t in range(AHEAD):
            x_prep(t)

        # ---- main loop ----
        for t in range(NT):
            if t + AHEAD < NT:
                x_prep(t + AHEAD)
            sc = ps_s.tile([PT, K], F32, tag="sc")
            s0, s1 = (slice(0, 512), slice(512, 1024))
            xh, xl = xs_tiles.pop(t)
            nc.tensor.matmul(sc[:, s0], xh[:], ch[:, s0], start=True, stop=False)
            nc.tensor.matmul(sc[:, s1], xh[:], ch[:, s1], start=True, stop=False)
            nc.tensor.matmul(sc[:, s0], xh[:], cl[:, s0], start=False, stop=False)
            nc.tensor.matmul(sc[:, s1], xh[:], cl[:, s1], start=False, stop=False)
            nc.tensor.matmul(sc[:, s0], xl[:], ch[:, s0], start=False, stop=False)
            nc.tensor.matmul(sc[:, s1], xl[:], ch[:, s1], start=False, stop=False)
            nc.tensor.matmul(sc[:, s0], ones2[:], negb2[:, s0],
                             start=False, stop=True)
            nc.tensor.matmul(sc[:, s1], ones2[:], negb2[:, s1],
                             start=False, stop=True)

            m = mcol_pool.tile([PT, 1], F32, tag="m")
            nc.vector.reduce_max(m[:], sc[:], axis=mybir.AxisListType.X)
            bias_col = mcol_pool.tile([PT, 1], F32, tag="bias")
            nc.gpsimd.tensor_scalar(bias_col[:], m[:], -BIG, lnmask[:, t:t + 1],
                                    op0=OP.mult, op1=OP.add)

            oh = oh_pool.tile([PT, K], F32)
            if t == NT - 1:
                # split the last tile so the final store drains in halves
                # on both queues (shorter kernel tail)
                nc.scalar.activation(oh[:, s0], sc[:, s0], AF.Exp,
                                     bias=bias_col[:], scale=BIG)
                nc.gpsimd.dma_start(out_t[t][:, s0],
                                    oh[:, s0])
                nc.scalar.activation(oh[:, s1], sc[:, s1], AF.Exp,
                                     bias=bias_col[:], scale=BIG)
                nc.sync.dma_start(out_t[t][:, s1],
                                  oh[:, s1])
            else:
                nc.scalar.activation(oh[:], sc[:], AF.Exp, bias=bias_col[:],
                                     scale=BIG)
                # alternate output stores across two DMA queues so per-DMA
                # fixed costs overlap (SWDGE/gpsimd and HWDGE/sync drain
                # independently)
                if t % 2 == 0:
                    nc.gpsimd.dma_start(out_t[t], oh[:])
                else:
                    nc.sync.dma_start(out_t[t], oh[:])


def _build():
    # Bacc (not raw Bass): its compile() moves matmul waits onto ldweights and
    # splits oversized wait lists into event-semaphore instructions — without
    # it walrus rejects Tile output with "Too many sync wait commands".
    nc = bacc.Bacc("TRN2", target_bir_lowering=False, debug=False, num_devices=B)
    x_in = nc.dram_tensor("x", [N, D], F32, kind="ExternalInput")
    mask_in = nc.dram_tensor("mask", [N], F32, kind="ExternalInput")
    c_in = nc.dram_tensor("cent", [K, D], F32, kind="ExternalInput")
    mean_in = nc.dram_tensor("mean", [D], F32, kind="ExternalInput")
    scale_in = nc.dram_tensor("scale", [D], F32, kind="ExternalInput")
    out = nc.dram_tensor("out", [N, K], F32, kind="ExternalOutput")
    with tile.TileContext(nc) as tc:
        _body(nc, tc, x_in[:], mask_in[:], c_in[:], mean_in[:], scale_in[:], out[:])
    nc.compile()
    return nc


_NC = None


def _run(inputs, trace=False, tmpdir=None):
    global _NC
    if _NC is None:
        _NC = _build()
    X = np.ascontiguousarray(inputs["X"], dtype=np.float32)
    mask = np.ascontiguousarray(inputs["mask"], dtype=np.float32)
    cent = np.ascontiguousarray(inputs["centroids"], dtype=np.float32)
    mean = np.ascontiguousarray(inputs["mean"], dtype=np.float32)
    scale = np.ascontiguousarray(inputs["scale"], dtype=np.float32)
    in_maps = [
        {"x": X[b], "mask": mask[b], "cent": cent, "mean": mean, "scale": scale}
        for b in range(B)
    ]
    res = run_bass_kernel_spmd(_NC, in_maps, list(range(B)), trace=trace,
                               tmpdir=tmpdir,
                               trace_cores=[0] if trace else None)
    full = np.stack([res.results[b]["out"] for b in range(B)], axis=0)
    return full, res


def kernel(**inputs) -> np.ndarray:
    full, _ = _run(inputs, trace=False)
    return full
